# revision 1
# baseline (speedup 1.0000x reference)
"""Trainium2 Bass kernel for nn_BaselineParser (segment-pool + transformer block +
biaffine parser loss), data-parallel over batch across 8 NeuronCores.

Self-contained: hardcodes shapes B=32, S=1024, D=768, F=2048, W=384, H=8.
Each core processes 4 batch rows and returns partial (sum nll*mask, sum mask);
the host combines partials into the scalar loss.

Numerics: matmul path runs in bf16 (weights folded/padded on host), the
"exact path" (masking, -1e9 fill, gold gather, log-softmax, final reductions)
runs in fp32.  The loss is dominated by gold-on-masked-column tokens whose
nll is ~1e9 computed exactly, so bf16 on the matmul path perturbs the loss
only at ~1e-6 relative.
"""

import math
import os
import numpy as np
import ml_dtypes

import concourse.bass as bass
import concourse.tile as tile
from concourse.tile import add_dep_helper
from concourse import bacc, mybir
from concourse.bass_utils import run_bass_kernel_spmd

F32 = mybir.dt.float32
BF16 = mybir.dt.bfloat16
I32 = mybir.dt.int32
AF = mybir.ActivationFunctionType
ALU = mybir.AluOpType
AX = mybir.AxisListType

B, S, D, FF = 32, 1024, 768, 2048
W = 384
H = 8
DH = 96
DHP = 128            # padded head dim
NCORES = 8
NB = B // NCORES     # batches per core
NEG = -1.0e9
KD = D // 128        # 6 contraction chunks over D
TC = W // 128        # 3 token chunks
SC = S // 128        # 8 subword chunks


# ---------------------------------------------------------------- host prep

def _prep_host(inp):
    """Fold LN scales + head padding into weight matrices (fp32 math, bf16 out)."""
    f4 = np.float32
    Wqkv = np.asarray(inp['Wqkv'], f4)
    bqkv = np.asarray(inp['bqkv'], f4)
    g1 = np.asarray(inp['ln1_g'], f4)
    b1ln = np.asarray(inp['ln1_b'], f4)
    g2 = np.asarray(inp['ln2_g'], f4)
    b2ln = np.asarray(inp['ln2_b'], f4)

    Wf = g1[:, None] * Wqkv                      # fold ln1 gain
    bf = b1ln @ Wqkv + bqkv                      # fold ln1 bias
    sc = f4(1.0 / math.sqrt(DH))
    Wf[:, :D] *= sc                              # fold 1/sqrt(dh) into Q
    bf[:D] *= sc

    # pad heads 96 -> 128: Q' heads 0..7, K' heads 8..15 -> [768, 2048]
    Wqk = np.zeros((D, 2 * H * DHP), f4)
    bqk = np.zeros((2 * H * DHP,), f4)
    for h in range(H):
        Wqk[:, DHP * h: DHP * h + DH] = Wf[:, DH * h: DH * h + DH]
        bqk[DHP * h: DHP * h + DH] = bf[DH * h: DH * h + DH]
        Wqk[:, DHP * (H + h): DHP * (H + h) + DH] = Wf[:, D + DH * h: D + DH * h + DH]
        bqk[DHP * (H + h): DHP * (H + h) + DH] = bf[D + DH * h: D + DH * h + DH]

    # V' [768, 1024]: head h cols 128h..128h+95, col 128h+96 is the all-ones
    # (colsum) column: zero weights, bias 1.
    Wv = np.zeros((D, H * DHP), f4)
    bv = np.zeros((H * DHP,), f4)
    for h in range(H):
        Wv[:, DHP * h: DHP * h + DH] = Wf[:, 2 * D + DH * h: 2 * D + DH * h + DH]
        bv[DHP * h: DHP * h + DH] = bf[2 * D + DH * h: 2 * D + DH * h + DH]
        bv[DHP * h + DH] = 1.0

    # Wo' [1024, 768]: rows 128h+j <- Wo rows 96h+j, pad rows zero.
    Wo = np.asarray(inp['Wo'], f4)
    Wop = np.zeros((H * DHP, D), f4)
    for h in range(H):
        Wop[DHP * h: DHP * h + DH] = Wo[DH * h: DH * h + DH]

    W1 = np.asarray(inp['W1'], f4)
    b1 = np.asarray(inp['b1'], f4)
    W1f = g2[:, None] * W1
    b1f = b2ln @ W1 + b1

    bf16 = ml_dtypes.bfloat16
    return {
        'wqk': Wqk.astype(bf16), 'bqk': bqk,
        'wv': Wv.astype(bf16), 'bv': bv.astype(bf16),
        'wo': Wop.astype(bf16), 'bo': np.asarray(inp['bo'], f4),
        'w1': W1f.astype(bf16), 'b1': b1f,
        'w2': np.asarray(inp['W2'], f4).astype(bf16),
        'b2': np.asarray(inp['b2'], f4),
        'wbi': np.asarray(inp['Wbi'], f4).astype(bf16),
        'uw': np.asarray(inp['Uw'], f4).astype(bf16),
        'ub': np.asarray(inp['Ub'], f4).reshape(1, 1),
        'root': np.asarray(inp['root'], f4).astype(bf16),
        'bo_bf': np.asarray(inp['bo'], f4).astype(bf16),
        'b2_bf': np.asarray(inp['b2'], f4).astype(bf16),
    }


# ---------------------------------------------------------------- bass build

def _declare(nc):
    """Declare per-core DRAM tensors; returns dict of APs."""
    t = {}

    def inp(name, shape, dt):
        t[name] = nc.dram_tensor(name, list(shape), dt, kind="ExternalInput").ap()

    inp('lh', (NB, S, D), BF16)
    inp('wid', (NB, S), I32)
    inp('gold', (NB, W), I32)
    inp('wqk', (D, 2 * H * DHP), BF16)
    inp('bqk', (2 * H * DHP,), F32)
    inp('wv', (D, H * DHP), BF16)
    inp('bv', (H * DHP,), BF16)
    inp('wo', (H * DHP, D), BF16)
    inp('bo', (D,), F32)
    inp('w1', (D, FF), BF16)
    inp('b1', (FF,), F32)
    inp('w2', (FF, D), BF16)
    inp('b2', (D,), F32)
    inp('wbi', (D, D), BF16)
    inp('uw', (D,), BF16)
    inp('ub', (1, 1), F32)
    inp('root', (D,), BF16)
    inp('bo_bf', (D,), BF16)
    inp('b2_bf', (D,), BF16)
    t['out'] = nc.dram_tensor('out', [1, 2], F32, kind="ExternalOutput").ap()
    return t


def _build_body(nc, tc_, t):
    """Emit the whole per-core program inside TileContext tc_."""
    import contextlib
    ctx = contextlib.ExitStack()
    with ctx:
        _build_body_inner(nc, tc_, t, ctx)


def _build_body_inner(nc, tc_, t, ctx):
    pool = ctx.enter_context
    con = pool(tc_.tile_pool(name="con", bufs=1))
    wbig = pool(tc_.tile_pool(name="wbig", bufs=6))
    wvp = pool(tc_.tile_pool(name="wvp", bufs=6))
    wst = pool(tc_.tile_pool(name="wst", bufs=17))
    lhp = pool(tc_.tile_pool(name="lhp", bufs=5))
    ohp = pool(tc_.tile_pool(name="ohp", bufs=8))
    xfam = pool(tc_.tile_pool(name="xfam", bufs=25))
    zp = pool(tc_.tile_pool(name="zp", bufs=12))
    sqp = pool(tc_.tile_pool(name="sqp", bufs=2))
    qkp = pool(tc_.tile_pool(name="qkp", bufs=3))
    vtp = pool(tc_.tile_pool(name="vtp", bufs=6))
    exp_p = pool(tc_.tile_pool(name="exp_p", bufs=3))
    yp = pool(tc_.tile_pool(name="yp", bufs=16))
    gp = pool(tc_.tile_pool(name="gp", bufs=2))
    t1p = pool(tc_.tile_pool(name="t1p", bufs=13))
    rows = pool(tc_.tile_pool(name="rows", bufs=4))
    batch_rows = pool(tc_.tile_pool(name="batch_rows", bufs=4))
    loss_p = pool(tc_.tile_pool(name="loss_p", bufs=2))
    bcp = pool(tc_.tile_pool(name="bcp", bufs=6))
    tmp_p = pool(tc_.tile_pool(name="tmp_p", bufs=2))

    ps_mm = pool(tc_.tile_pool(name="ps_mm", bufs=2, space="PSUM"))
    ps_acc = pool(tc_.tile_pool(name="ps_acc", bufs=6, space="PSUM"))

    # ---------------- constants
    ones_col = con.tile([128, 1], BF16)
    nc.gpsimd.memset(ones_col[:], 1.0)
    ones_row = con.tile([1, 128], BF16)
    nc.gpsimd.memset(ones_row[:], 1.0)
    ones_col_f = con.tile([128, 1], F32)
    nc.gpsimd.memset(ones_col_f[:], 1.0)
    ones_row384 = con.tile([1, W], BF16)
    nc.gpsimd.memset(ones_row384[:], 1.0)

    iota_w = con.tile([128, W], I32)
    nc.gpsimd.iota(iota_w[:], pattern=[[1, W]], base=0, channel_multiplier=0)
    iota385_i = loss_p.tile([128, W + 1], I32, name="iota385_i", tag="e1", bufs=2)
    nc.gpsimd.iota(iota385_i[:], pattern=[[1, W + 1]], base=0, channel_multiplier=0)
    iota385_f = con.tile([128, W + 1], F32)
    nc.vector.tensor_copy(iota385_f[:], iota385_i[:])
    iotam1_i = loss_p.tile([1, W + 1], I32, name="iotam1_i", tag="e1", bufs=2)
    nc.gpsimd.iota(iotam1_i[:], pattern=[[1, W + 1]], base=-1, channel_multiplier=0)
    iotam1_f = con.tile([1, W + 1], F32)
    nc.vector.tensor_copy(iotam1_f[:], iotam1_i[:])
    iota_p = []
    for c in range(TC):
        ip_i = tmp_p.tile([128, 1], I32, name=f"ip_i{c}", tag="ip_i")
        nc.gpsimd.iota(ip_i[:], pattern=[[0, 1]], base=128 * c, channel_multiplier=1)
        ip_f = con.tile([128, 1], F32, name=f"ip_f{c}", tag=f"ip_f{c}")
        nc.vector.tensor_copy(ip_f[:], ip_i[:])
        iota_p.append(ip_f)

    NM12 = con.tile([128, NB * TC], F32)
    M12 = con.tile([128, NB * TC], F32)

    X = [[None] * KD for _ in range(NB)]
    cneg_b = [None] * NB
    gold_f = [None] * NB
    ln1_st = [None] * NB

    # ================ helper: LN split into stats + apply ================
    def ln_stats(xt, b, label):
        s1 = ps_acc.tile([1, W], F32, name=f"s1{label}{b}", tag="ps_acc")
        for k in range(KD):
            nc.tensor.matmul(s1[:], lhsT=ones_col[:], rhs=xt[k][:],
                             start=(k == 0), stop=(k == KD - 1))
        s2 = ps_acc.tile([1, W], F32, name=f"s2{label}{b}", tag="ps_acc")
        for k in range(KD):
            sq = sqp.tile([128, W], BF16, name=f"sq{label}{b}_{k}", tag="sq")
            nc.scalar.activation(sq[:], xt[k][:], AF.Square)
            nc.tensor.matmul(s2[:], lhsT=ones_col[:], rhs=sq[:],
                             start=(k == 0), stop=(k == KD - 1))
        mean = rows.tile([1, W], F32, name=f"mean{label}{b}", tag="lnrow", bufs=5)
        nc.vector.tensor_scalar_mul(mean[:], s1[:], 1.0 / D)
        v = rows.tile([1, W], F32, name=f"v{label}{b}", tag="lnrow", bufs=5)
        nc.vector.tensor_scalar_mul(v[:], s2[:], 1.0 / D)
        m2 = rows.tile([1, W], F32, name=f"m2{label}{b}", tag="lnrow", bufs=5)
        nc.vector.tensor_tensor(out=m2[:], in0=mean[:], in1=mean[:], op=ALU.mult)
        nc.vector.tensor_tensor(out=v[:], in0=v[:], in1=m2[:], op=ALU.subtract)
        nc.vector.tensor_scalar_add(v[:], v[:], 1e-5)
        r = rows.tile([1, W], F32, name=f"r{label}{b}", tag="lnrow", bufs=5)
        nc.vector.reciprocal_approx_fast(out=r[:], in_=v[:])
        rstd = rows.tile([1, W], F32, name=f"rstd{label}{b}", tag="lnrow", bufs=5)
        nc.scalar.activation(rstd[:], r[:], AF.Sqrt)
        nc.vector.tensor_tensor(out=mean[:], in0=mean[:], in1=rstd[:], op=ALU.mult)
        rstd_b = bcp.tile([128, W], F32, name=f"rstdB{label}{b}", tag="bc", bufs=12)
        nc.gpsimd.partition_broadcast(rstd_b[:], rstd[:])
        mpr_b = bcp.tile([128, W], F32, name=f"mprB{label}{b}", tag="bc", bufs=12)
        nc.gpsimd.partition_broadcast(mpr_b[:], mean[:])
        return rstd_b, mpr_b

    def ln_apply(xt, b, label, st):
        rstd_b, mpr_b = st
        z = []
        for k in range(KD):
            zt = zp.tile([128, W], BF16, name=f"z{label}{b}_{k}", tag="z")
            tt = tmp_p.tile([128, W], BF16, name=f"zt{label}{b}_{k}", tag="ztmp")
            nc.vector.tensor_tensor(out=tt[:], in0=xt[k][:], in1=rstd_b[:], op=ALU.mult)
            nc.vector.tensor_tensor(out=zt[:], in0=tt[:], in1=mpr_b[:], op=ALU.subtract)
            z.append(zt)
        return z

    def emit_v(b, z):
        vt = []
        for c in range(TC):
            v_ = vtp.tile([128, H * DHP], BF16, name=f"V{b}_{c}", tag="vt")
            for n in range(2):
                cs = slice(512 * n, 512 * (n + 1))
                vp = ps_mm.tile([128, 512], F32, name=f"vp{b}_{c}_{n}", tag="ps_mm")
                for k in range(KD):
                    nc.tensor.matmul(vp[:], lhsT=z[k][:, 128 * c:128 * (c + 1)],
                                     rhs=wv_t[k][:, cs], start=(k == 0), stop=False)
                nc.tensor.matmul(vp[:], lhsT=ones_row[:], rhs=bv_row[:, cs],
                                 start=False, stop=True)
                nc.scalar.copy(v_[:, cs], vp[:])
            vt.append(v_)
        return vt

    def emit_heads(b, z, vt):
        y = []
        for h in range(H):
            qk = []
            for m in (h, H + h):
                qp = ps_mm.tile([128, W], F32, name=f"qp{b}_{m}", tag="ps_mm")
                for k in range(KD):
                    nc.tensor.matmul(qp[:], lhsT=wqk_t[k][:, 128 * m:128 * (m + 1)],
                                     rhs=z[k][:], start=(k == 0), stop=(k == KD - 1))
                qs = qkp.tile([128, W], BF16, name=f"qk{b}_{m}", tag="qk")
                nc.scalar.activation(qs[:], qp[:], AF.Identity,
                                     bias=bias['bqk'][:, m:m + 1])
                qk.append(qs)
            q_t, k_t = qk

            ex = []
            for c in range(TC):
                sp = ps_acc.tile([128, W], F32, name=f"sp{b}_{h}_{c}", tag="ps_acc")
                nc.tensor.matmul(sp[:], lhsT=k_t[:, 128 * c:128 * (c + 1)],
                                 rhs=q_t[:], start=True, stop=True)
                e_ = exp_p.tile([128, W], BF16, name=f"ex{b}_{h}_{c}", tag="ex")
                nc.scalar.activation(e_[:], sp[:], AF.Exp)
                ex.append(e_)

            yraw = ps_acc.tile([128, W], F32, name=f"yraw{b}_{h}", tag="ps_acc")
            for c in range(TC):
                nc.tensor.matmul(yraw[:], lhsT=vt[c][:, DHP * h:DHP * (h + 1)],
                                 rhs=ex[c][:], start=(c == 0), stop=(c == TC - 1))
            csr = rows.tile([1, W], F32, name=f"csr{b}_{h}", tag="rowf")
            nc.vector.tensor_copy(csr[:], yraw[DH:DH + 1, :])
            rcp = rows.tile([1, W], F32, name=f"arcp{b}_{h}", tag="rowf")
            nc.vector.reciprocal_approx_fast(out=rcp[:], in_=csr[:])
            rb = bcp.tile([128, W], F32, name=f"arb{b}_{h}", tag="bc", bufs=12)
            nc.gpsimd.partition_broadcast(rb[:], rcp[:])
            y_ = yp.tile([128, W], BF16, name=f"y{b}_{h}", tag="y")
            nc.vector.tensor_tensor(out=y_[:], in0=yraw[:], in1=rb[:], op=ALU.mult)
            y.append(y_)
        return y

    def emit_wo(b, y, wo_t):
        for m in range(KD):
            op = ps_mm.tile([128, W], F32, name=f"op{b}_{m}", tag="ps_mm")
            for k in range(H):
                nc.tensor.matmul(op[:], lhsT=wo_t[k][:, 128 * m:128 * (m + 1)],
                                 rhs=y[k][:], start=(k == 0), stop=False)
            nc.tensor.matmul(op[:], lhsT=bo_row[:, 128 * m:128 * (m + 1)],
                             rhs=ones_row384[:], start=False, stop=True)
            x2 = xfam.tile([128, W], BF16, name=f"X2_{b}_{m}", tag="xfam")
            last = nc.vector.tensor_tensor(out=x2[:], in0=op[:], in1=X[b][m][:], op=ALU.add)
            X2[b][m] = x2
        return last

    # ================ P0: pool (segment mean), s-outer ================
    sums = []
    for d in range(KD):
        sums.append(ps_acc.tile([128, W], F32, name=f"sums{d}", tag="ps_acc"))
    for b in range(NB):
        wid_i = tmp_p.tile([128, SC], I32, name=f"wid_i{b}", tag="wid_i")
        nc.sync.dma_start(wid_i[:], t['wid'][b].rearrange("(c p) -> p c", p=128))
        mx_i = tmp_p.tile([1, 1], I32, name=f"mx_i{b}", tag="mx_i")
        nc.sync.dma_start(mx_i[:], t['wid'][b:b + 1, S - 1:S])
        mx_f = tmp_p.tile([1, 1], F32, name=f"mx_f{b}", tag="mx_f")
        nc.vector.tensor_copy(mx_f[:], mx_i[:])

        g_i = tmp_p.tile([128, TC], I32, name=f"g_i{b}", tag="g_i")
        nc.sync.dma_start(g_i[:], t['gold'][b].rearrange("(c p) -> p c", p=128))
        gf = batch_rows.tile([128, TC], F32, name=f"gold_f{b}", tag="gold_f")
        nc.vector.tensor_copy(gf[:], g_i[:])
        gold_f[b] = gf

        cnts = ps_mm.tile([1, W], F32, name=f"cnts{b}", tag="ps_mm")
        lh_t, oh_t = [], []
        for s in range(SC):
            lh_ = lhp.tile([128, D], BF16, name=f"lh{b}_{s}", tag="lh", bufs=5)
            nc.sync.dma_start(lh_[:], t['lh'][b, 128 * s:128 * (s + 1), :])
            lh_t.append(lh_)
            oh_ = ohp.tile([128, W], BF16, name=f"oh{b}_{s}", tag="oh", bufs=8)
            nc.vector.tensor_tensor(
                out=oh_[:], in0=wid_i[:, s:s + 1].to_broadcast([128, W]),
                in1=iota_w[:], op=ALU.is_equal)
            oh_t.append(oh_)
            nc.tensor.matmul(cnts[:], lhsT=ones_col[:], rhs=oh_[:],
                             start=(s == 0), stop=(s == SC - 1))
        for s in range(SC):
            for d in range(KD):
                nc.tensor.matmul(sums[d][:], lhsT=lh_t[s][:, 128 * d:128 * (d + 1)],
                                 rhs=oh_t[s][:], start=(s == 0), stop=(s == SC - 1))

        c1 = rows.tile([1, W], F32, name=f"c1_{b}", tag="rowf")
        nc.vector.tensor_scalar_max(c1[:], cnts[:], 1.0)
        rcp = rows.tile([1, W], F32, name=f"rcp{b}", tag="rowf")
        nc.vector.reciprocal_approx_fast(out=rcp[:], in_=c1[:])
        rb = bcp.tile([128, W], F32, name=f"rb{b}", tag="bc", bufs=12)
        nc.gpsimd.partition_broadcast(rb[:], rcp[:])
        for d in range(KD):
            x_ = xfam.tile([128, W], BF16, name=f"X{b}_{d}", tag="xfam")
            nc.vector.tensor_tensor(out=x_[:], in0=sums[d][:], in1=rb[:], op=ALU.mult)
            X[b][d] = x_

        maxid = tmp_p.tile([128, 1], F32, name=f"maxid{b}", tag="maxid")
        nc.gpsimd.partition_broadcast(maxid[:], mx_f[:])
        for c in range(TC):
            nc.vector.tensor_tensor(out=M12[:, TC * b + c:TC * b + c + 1],
                                    in0=iota_p[c][:], in1=maxid[:], op=ALU.is_le)
        ct = rows.tile([1, W + 1], F32, name=f"ct{b}", tag="rowf")
        nc.vector.tensor_tensor(out=ct[:], in0=iotam1_f[:],
                                in1=maxid[0:1, 0:1].to_broadcast([1, W + 1]),
                                op=ALU.is_gt)
        cr = rows.tile([1, W + 1], F32, name=f"cr{b}", tag="rowf")
        nc.vector.tensor_scalar_mul(cr[:], ct[:], NEG)
        cb = batch_rows.tile([128, W + 1], F32, name=f"cneg{b}", tag="cneg")
        nc.gpsimd.partition_broadcast(cb[:], cr[:])
        cneg_b[b] = cb

    # ---------------- weights / biases (after P0 so lh DMAs go first)
    wqk_t = []
    for k in range(KD):
        w_ = wbig.tile([128, 2 * H * DHP], BF16, name=f"wqk{k}", tag="wbig")
        nc.sync.dma_start(w_[:], t['wqk'][128 * k:128 * (k + 1), :])
        wqk_t.append(w_)
    wv_t = []
    for k in range(KD):
        w_ = wvp.tile([128, H * DHP], BF16, name=f"wv{k}", tag="wv")
        nc.sync.dma_start(w_[:], t['wv'][128 * k:128 * (k + 1), :])
        wv_t.append(w_)

    bias = {}
    for name, n, dt in (('bqk', 16, F32), ('b1', 16, F32), ('bo', 6, F32),
                        ('b2', 6, F32), ('root', 6, BF16), ('uw', 6, BF16)):
        b_ = con.tile([128, n], dt, name=f"bc_{name}", tag=f"bc_{name}")
        nc.sync.dma_start(b_[:], t[name].rearrange("(n p) -> p n", p=128))
        bias[name] = b_
    bv_row = con.tile([1, H * DHP], BF16)
    nc.sync.dma_start(bv_row[:], t['bv'][None, :])
    bo_row = con.tile([1, D], BF16)
    nc.sync.dma_start(bo_row[:], t['bo_bf'][None, :])
    b2_row = con.tile([1, D], BF16)
    nc.sync.dma_start(b2_row[:], t['b2_bf'][None, :])
    ub_t = con.tile([1, 1], F32)
    nc.sync.dma_start(ub_t[:], t['ub'][:, :])

    # ================ P1-P4 in batch pairs ================
    wo_t = []
    for k in range(H):
        w_ = wst.tile([128, D], BF16, name=f"wo{k}", tag="wst")
        nc.sync.dma_start(w_[:], t['wo'][128 * k:128 * (k + 1), :])
        wo_t.append(w_)
    X2 = [[None] * KD for _ in range(NB)]
    ln2_st = [None] * NB
    for b0 in range(0, NB, 2):
        b1 = b0 + 1
        stA = ln1_st[b0] if ln1_st[b0] is not None else ln_stats(X[b0], b0, "A")
        stB = ln1_st[b1] if ln1_st[b1] is not None else ln_stats(X[b1], b1, "A")
        zA = ln_apply(X[b0], b0, "A", stA)
        zB = ln_apply(X[b1], b1, "A", stB)
        vA = emit_v(b0, zA)
        vB = emit_v(b1, zB)
        yA = emit_heads(b0, zA, vA)
        yB = emit_heads(b1, zB, vB)
        emit_wo(b0, yA, wo_t)
        ln2_st[b0] = ln_stats(X2[b0], b0, "B")
        m_p4 = emit_wo(b1, yB, wo_t)
        ln2_st[b1] = ln_stats(X2[b1], b1, "B")

    # ================ P5: LN2 + FFN in batch pairs ================
    w1_t = []
    for k in range(KD):
        w_ = wbig.tile([128, FF], BF16, name=f"w1_{k}", tag="wbig")
        nc.sync.dma_start(w_[:], t['w1'][128 * k:128 * (k + 1), :])
        w1_t.append(w_)
    w2_t = []
    for m in range(FF // 128):
        w_ = wst.tile([128, D], BF16, name=f"w2_{m}", tag="wst")
        dma = nc.sync.dma_start(w_[:], t['w2'][128 * m:128 * (m + 1), :])
        add_dep_helper(dma.ins, m_p4.ins, reason="w2 load after P4 frees wst")
        w2_t.append(w_)

    X3 = [[None] * KD for _ in range(NB)]
    x3p = []
    for m2 in range(KD):
        x3p.append(ps_acc.tile([128, W], F32, name=f"x3p{m2}", tag="ps_acc"))

    def emit_ffn(b, z2):
        for m in range(FF // 128):
            wp = ps_mm.tile([128, W], F32, name=f"wp{b}_{m}", tag="ps_mm")
            for k in range(KD):
                mm = nc.tensor.matmul(wp[:], lhsT=w1_t[k][:, 128 * m:128 * (m + 1)],
                                 rhs=z2[k][:], start=(k == 0), stop=(k == KD - 1))
                if k == 0:
                    add_dep_helper(mm.ins, m_p4.ins, reason="ffn after P4")
            g_ = gp.tile([128, W], BF16, name=f"G{b}_{m}", tag="g")
            nc.scalar.activation(g_[:], wp[:], AF.Gelu, bias=bias['b1'][:, m:m + 1])
            for m2 in range(KD):
                nc.tensor.matmul(x3p[m2][:], lhsT=w2_t[m][:, 128 * m2:128 * (m2 + 1)],
                                 rhs=g_[:], start=(m == 0), stop=False)
        for m2 in range(KD):
            nc.tensor.matmul(x3p[m2][:], lhsT=b2_row[:, 128 * m2:128 * (m2 + 1)],
                             rhs=ones_row384[:], start=False, stop=True)
            x3 = xfam.tile([128, W], BF16, name=f"X3_{b}_{m2}", tag="xfam")
            last = nc.vector.tensor_tensor(out=x3[:], in0=x3p[m2][:], in1=X2[b][m2][:], op=ALU.add)
            X3[b][m2] = x3
        return last

    for b0 in range(0, NB, 2):
        b1 = b0 + 1
        z2A = ln_apply(X2[b0], b0, "B", ln2_st[b0])
        emit_ffn(b0, z2A)
        z2B = ln_apply(X2[b1], b1, "B", ln2_st[b1])
        m_p5 = emit_ffn(b1, z2B)

    # ================ P6-P7: biaffine + loss in batch pairs ================
    wbi_t = []
    for k in range(KD):
        w_ = vtp.tile([128, H * DHP], BF16, name=f"wbi{k}", tag="vt")
        nc.sync.dma_start(w_[:, 0:D], t['wbi'][128 * k:128 * (k + 1), :])
        wbi_t.append(w_)

    def emit_t1_u(b):
        t1 = []
        for m in range(KD):
            bp = ps_mm.tile([128, W], F32, name=f"bp{b}_{m}", tag="ps_mm")
            for k in range(KD):
                mm = nc.tensor.matmul(bp[:], lhsT=wbi_t[k][:, 128 * m:128 * (m + 1)],
                                 rhs=X3[b][k][:], start=(k == 0), stop=(k == KD - 1))
                if k == 0:
                    add_dep_helper(mm.ins, m_p5.ins, reason="bil after P5")
            t1_ = t1p.tile([128, W], BF16, name=f"T1_{b}_{m}", tag="t1")
            nc.scalar.copy(t1_[:], bp[:])
            t1.append(t1_)
        up0 = ps_mm.tile([1, 1], F32, name=f"up0{b}", tag="ps_mm")
        for k in range(KD):
            nc.tensor.matmul(up0[:], lhsT=bias['uw'][:, k:k + 1],
                             rhs=bias['root'][:, k:k + 1],
                             start=(k == 0), stop=(k == KD - 1))
        upx = ps_mm.tile([1, W], F32, name=f"upx{b}", tag="ps_mm")
        for k in range(KD):
            nc.tensor.matmul(upx[:], lhsT=bias['uw'][:, k:k + 1],
                             rhs=X3[b][k][:], start=(k == 0), stop=(k == KD - 1))
        u_f = rows.tile([1, W + 1], F32, name=f"uf{b}", tag="rowf")
        nc.vector.tensor_scalar_add(u_f[:, 0:1], up0[:], ub_t[0:1, 0:1])
        nc.vector.tensor_scalar_add(u_f[:, 1:W + 1], upx[:], ub_t[0:1, 0:1])
        u_bf = rows.tile([1, W + 1], BF16, name=f"ubf{b}", tag="rowb", bufs=2)
        nc.vector.tensor_copy(u_bf[:], u_f[:])
        return t1, u_bf

    def emit_loss(b, t1, u_bf):
        Lms, mxs, Ss = [], [], []
        for c in range(TC):
            L = ps_acc.tile([128, W + 1], F32, name=f"L{b}_{c}", tag="ps_acc")
            nc.tensor.matmul(L[:, :], lhsT=ones_row[:], rhs=u_bf[:],
                             start=True, stop=False)
            for k in range(KD):
                nc.tensor.matmul(L[:, 0:1], lhsT=t1[k][:, 128 * c:128 * (c + 1)],
                                 rhs=bias['root'][:, k:k + 1],
                                 start=False, stop=False)
            for k in range(KD):
                nc.tensor.matmul(L[:, 1:W + 1],
                                 lhsT=t1[k][:, 128 * c:128 * (c + 1)],
                                 rhs=X3[b][k][:], start=False, stop=(k == KD - 1))
            Lm = loss_p.tile([128, W + 1], F32, name=f"Lm{b}_{c}", tag="lm", bufs=4)
            nc.vector.tensor_tensor(out=Lm[:], in0=L[:], in1=cneg_b[b][:], op=ALU.add)
            nmx = rows.tile([128, 1], F32, name=f"nmx{b}_{c}", tag="colf", bufs=12)
            nc.vector.tensor_reduce(out=nmx[:], in_=Lm[:], axis=AX.X, op=ALU.max,
                                    negate=True)
            E = loss_p.tile([128, W + 1], F32, name=f"E{b}_{c}", tag="e1", bufs=2)
            Ssum = rows.tile([128, 1], F32, name=f"S{b}_{c}", tag="colf", bufs=12)
            nc.scalar.activation(E[:], Lm[:], AF.Exp, bias=nmx[:], accum_out=Ssum[:])
            Lms.append(Lm)
            mxs.append(nmx)
            Ss.append(Ssum)
        lnSs = []
        for c in range(TC):
            lnS = rows.tile([128, 1], F32, name=f"lnS{b}_{c}", tag="colf", bufs=12)
            nc.scalar.activation(lnS[:], Ss[c][:], AF.Ln)
            lnSs.append(lnS)
        for c in range(TC):
            oneh = loss_p.tile([128, W + 1], F32, name=f"oneh{b}_{c}", tag="lm", bufs=4)
            nc.vector.tensor_tensor(
                out=oneh[:], in0=iota385_f[:],
                in1=gold_f[b][:, c:c + 1].to_broadcast([128, W + 1]), op=ALU.is_equal)
            E2 = loss_p.tile([128, W + 1], F32, name=f"E2{b}_{c}", tag="e1", bufs=2)
            picked = rows.tile([128, 1], F32, name=f"pk{b}_{c}", tag="colf", bufs=12)
            nc.vector.tensor_tensor(out=E2[:], in0=Lms[c][:], in1=oneh[:], op=ALU.mult)
            nc.vector.tensor_reduce(out=picked[:], in_=E2[:], axis=AX.X, op=ALU.add)
            t_ = rows.tile([128, 1], F32, name=f"nt{b}_{c}", tag="colf", bufs=12)
            nc.vector.tensor_tensor(out=t_[:], in0=lnSs[c][:], in1=picked[:],
                                    op=ALU.subtract)
            nll = rows.tile([128, 1], F32, name=f"nll{b}_{c}", tag="colf", bufs=12)
            nc.vector.tensor_tensor(out=nll[:], in0=t_[:], in1=mxs[c][:],
                                    op=ALU.subtract)
            j = TC * b + c
            nc.vector.tensor_tensor(out=NM12[:, j:j + 1], in0=nll[:],
                                    in1=M12[:, j:j + 1], op=ALU.mult)

    for b0 in range(0, NB, 2):
        b1 = b0 + 1
        t1A, uA = emit_t1_u(b0)
        t1B, uB = emit_t1_u(b1)
        emit_loss(b0, t1A, uA)
        emit_loss(b1, t1B, uB)

    # ================ P8: final reduction (exact fp32 matmul) ================
    out_sb = con.tile([1, 2], F32)
    fp1 = ps_mm.tile([1, NB * TC], F32, name="fp1", tag="ps_mm")
    nc.tensor.matmul(fp1[:], lhsT=ones_col_f[:], rhs=NM12[:], start=True, stop=True)
    nc.vector.tensor_reduce(out=out_sb[:, 0:1], in_=fp1[:], axis=AX.X, op=ALU.add)
    fp2 = ps_mm.tile([1, NB * TC], F32, name="fp2", tag="ps_mm")
    nc.tensor.matmul(fp2[:], lhsT=ones_col_f[:], rhs=M12[:], start=True, stop=True)
    nc.vector.tensor_reduce(out=out_sb[:, 1:2], in_=fp2[:], axis=AX.X, op=ALU.add)
    nc.sync.dma_start(t['out'][:, :], out_sb[:])


# ---------------------------------------------------------------- driver

_CACHE = {}


def build_nc():
    if 'nc' in _CACHE:
        return _CACHE['nc']
    nc = bacc.Bacc("TRN2", target_bir_lowering=False, debug=False)
    t = _declare(nc)
    with tile.TileContext(nc) as tc_:
        _build_body(nc, tc_, t)
    nc.compile()
    _CACHE['nc'] = nc
    return nc


def kernel(**inputs):
    nc = build_nc()
    host = _prep_host(inputs)
    bf16 = ml_dtypes.bfloat16
    lh = np.asarray(inputs['last_hidden'], np.float32).astype(bf16)
    wid = np.asarray(inputs['word_ids'], np.int32)
    gold = np.asarray(inputs['heads_gold'], np.int32)

    in_maps = []
    for c in range(NCORES):
        sl = slice(c * NB, (c + 1) * NB)
        m = {'lh': lh[sl], 'wid': wid[sl], 'gold': gold[sl]}
        m.update(host)
        in_maps.append(m)

    res = run_bass_kernel_spmd(nc, in_maps, core_ids=list(range(NCORES)))
    num = 0.0
    den = 0.0
    for c in range(NCORES):
        o = res.results[c]['out']
        num += float(o[0, 0])
        den += float(o[0, 1])
    return np.float32(num / den)


if __name__ == '__main__':
    build_nc()
    print("build + compile OK")



# revision 18
# speedup vs baseline: 1.0675x; 1.0675x over previous
"""Trainium2 Bass kernel for nn_BaselineParser — fp8 DoubleRow rewrite.

Data-parallel over batch across 8 cores (4 rows/core). All heavy matmuls run
in fp8e4 with DoubleRow perf mode (K=256 per instruction, 2x bf16 rate).
Token dim of the 4 rows is concatenated (T=1536) so weight-stationary matmuls
batch across rows.

Scale scheme (powers of 2, exact):
  weights x16 fp8, residual stream X/X2/X3 x16 fp8, v x16 fp8,
  z/q/k/ex/y/g/t1 x1 fp8, logits psum = 16*logits,
  cneg_u = 16*(u + NEG*mask).  LN is scale-invariant; rescales ride on
  activation(scale=) and scalar_tensor_tensor scalars.

Loss structure makes fp8 safe: the total is dominated by gold-on-masked
tokens contributing exactly 1e9 each (int-exact mask logic); the fp8 network
path only perturbs the ~1e-5-relative remainder.
"""

import math
import os
import numpy as np
import ml_dtypes

import concourse.bass as bass
import concourse.tile as tile
from concourse import bacc, mybir
from concourse.bass_utils import run_bass_kernel_spmd

F32 = mybir.dt.float32
BF16 = mybir.dt.bfloat16
FP8 = mybir.dt.float8e4
I32 = mybir.dt.int32
AF = mybir.ActivationFunctionType
ALU = mybir.AluOpType
AX = mybir.AxisListType
DR = mybir.MatmulPerfMode.DoubleRow

B, S, D, FF = 32, 1024, 768, 2048
W = 384
H = 8
DH = 96
NCORES = 8
NB = B // NCORES          # 4 rows per core
T = NB * W                # 1536 batched tokens
TA = NB * (W + 1)         # 1540 with root cols
KP = D // 256             # 3 d-pairs
SP = S // 256             # 4 subword-pairs
FP = FF // 256            # 8 ff-pairs
SC = 16.0                 # global power-of-2 scale
NEG16 = -16.0e9
NP8 = ml_dtypes.float8_e4m3


# ---------------------------------------------------------------- host prep

def _pairs(a):
    """[K, N] -> [K//256, 128, 2, N] DoubleRow interleave."""
    K, N = a.shape
    return np.ascontiguousarray(a.reshape(K // 256, 2, 128, N).transpose(0, 2, 1, 3))


def _prep_host(inp):
    f4 = np.float32
    Wqkv = np.asarray(inp['Wqkv'], f4)
    g1 = np.asarray(inp['ln1_g'], f4)
    b1ln = np.asarray(inp['ln1_b'], f4)
    Wf = g1[:, None] * Wqkv
    bf = b1ln @ Wqkv + np.asarray(inp['bqkv'], f4)
    scq = f4(1.0 / math.sqrt(DH))
    Wf[:, :D] *= scq
    bf[:D] *= scq

    # QK: head-padded 96->128, slots q0..q7,k0..k7 -> [768, 2048], x16
    Wqk = np.zeros((D, 2 * H * 128), f4)
    bqk = np.zeros((2 * H * 128,), f4)
    for h in range(H):
        Wqk[:, 128 * h:128 * h + DH] = Wf[:, DH * h:DH * h + DH]
        bqk[128 * h:128 * h + DH] = bf[DH * h:DH * h + DH]
        Wqk[:, 128 * (H + h):128 * (H + h) + DH] = Wf[:, D + DH * h:D + DH * h + DH]
        bqk[128 * (H + h):128 * (H + h) + DH] = bf[D + DH * h:D + DH * h + DH]

    # V: 97-packed heads (col 97h+96 is the denom ones-column) -> [768, 784]
    VW = 784  # 776 padded to a 16-multiple for DoubleRow lhsT step rule
    Wv = np.zeros((D, VW), f4)
    bv16 = np.zeros((VW,), f4)
    for h in range(H):
        Wv[:, 97 * h:97 * h + DH] = Wf[:, 2 * D + DH * h:2 * D + DH * h + DH]
        bv16[97 * h:97 * h + DH] = SC * bf[2 * D + DH * h:2 * D + DH * h + DH]
        bv16[97 * h + 96] = SC

    # Wo: 128-padded head rows -> [1024, 768]
    Wo = np.asarray(inp['Wo'], f4)
    Wop = np.zeros((H * 128, D), f4)
    for h in range(H):
        Wop[128 * h:128 * h + DH] = Wo[DH * h:DH * h + DH]

    g2 = np.asarray(inp['ln2_g'], f4)
    b2ln = np.asarray(inp['ln2_b'], f4)
    W1 = np.asarray(inp['W1'], f4)
    W1f = g2[:, None] * W1
    b1f = b2ln @ W1 + np.asarray(inp['b1'], f4)

    q8 = lambda x: (x * SC).astype(NP8)
    bfl = ml_dtypes.bfloat16
    return {
        'wqk': _pairs(q8(Wqk)),
        'wv': _pairs(q8(Wv)),
        'wo': _pairs(q8(Wop)),
        'w1': _pairs(q8(W1f)),
        'w2': _pairs(q8(np.asarray(inp['W2'], f4))),
        'wbi': _pairs(q8(np.asarray(inp['Wbi'], f4))),
        'uw': _pairs(np.pad(q8(np.asarray(inp['Uw'], f4))[:, None].astype(np.float32),
                            ((0, 0), (0, 15))).astype(NP8)),
        'root': _pairs(q8(np.asarray(inp['root'], f4))[:, None]),
        'bqk': np.ascontiguousarray(bqk.reshape(16, 128).T),        # [128,16] f32
        'b1c': np.ascontiguousarray(b1f.reshape(16, 128).T),        # [128,16] f32
        'bv16': (bv16).astype(bfl)[None, :],                        # [1,776]
        'bo16': (SC * np.asarray(inp['bo'], f4)).astype(bfl)[None, :],
        'b216': (SC * np.asarray(inp['b2'], f4)).astype(bfl)[None, :],
        'ub16': (SC * np.asarray(inp['Ub'], f4)).reshape(1, 1),
    }


def make_in_maps(inputs):
    host = _prep_host(inputs)
    lh8 = np.asarray(inputs['last_hidden'], np.float32).astype(NP8)
    # [B,S,D] -> per-core [NB, SP, 128, 2, D] DoubleRow interleave over s
    lh8 = np.ascontiguousarray(
        lh8.reshape(B, SP, 2, 128, D).transpose(0, 1, 3, 2, 4))
    wid = np.asarray(inputs['word_ids'], np.int32)
    gold = np.asarray(inputs['heads_gold'], np.int32)
    maps = []
    for c in range(NCORES):
        sl = slice(c * NB, (c + 1) * NB)
        m = {'lh': lh8[sl], 'wid': wid[sl], 'gold': gold[sl]}
        m.update(host)
        maps.append(m)
    return maps


# ---------------------------------------------------------------- bass build

def _declare(nc):
    t = {}

    def inp(name, shape, dt):
        t[name] = nc.dram_tensor(name, list(shape), dt, kind="ExternalInput").ap()

    inp('lh', (NB, SP, 128, 2, D), FP8)
    inp('wid', (NB, S), I32)
    inp('gold', (NB, W), I32)
    inp('wqk', (KP, 128, 2, 2048), FP8)
    inp('wv', (KP, 128, 2, 784), FP8)
    inp('wo', (4, 128, 2, D), FP8)
    inp('w1', (KP, 128, 2, FF), FP8)
    inp('w2', (FP, 128, 2, D), FP8)
    inp('wbi', (KP, 128, 2, D), FP8)
    inp('uw', (KP, 128, 2, 16), FP8)
    inp('root', (KP, 128, 2, 1), FP8)
    inp('bqk', (128, 16), F32)
    inp('b1c', (128, 16), F32)
    inp('bv16', (1, 784), BF16)
    inp('bo16', (1, D), BF16)
    inp('b216', (1, D), BF16)
    inp('ub16', (1, 1), F32)
    t['out'] = nc.dram_tensor('out', [1, 2], F32, kind="ExternalOutput").ap()
    if os.environ.get('KDBG'):
        for name, shape, dt in [
                ('dbg_x', (128, 2, T), FP8), ('dbg_z', (128, 2, T), FP8),
                ('dbg_q', (128, T), FP8), ('dbg_k', (128, T), FP8),
                ('dbg_v', (128, 2, 784), FP8), ('dbg_ex', (128, 2, W), FP8),
                ('dbg_y', (128, 2, T), FP8), ('dbg_x2', (128, 2, T), FP8),
                ('dbg_x3', (128, 2, 1552), FP8), ('dbg_t1', (128, 2, T), FP8),
                ('dbg_lm', (128, W + 1), F32), ('dbg_nm', (128, NB * 3), F32),
                ('dbg_u', (1, TA), F32)]:
            t[name] = nc.dram_tensor(name, list(shape), dt,
                                     kind="ExternalOutput").ap()
    return t


def _build_body(nc, tc_, t):
    import contextlib
    ctx = contextlib.ExitStack()
    with ctx:
        _build_body_inner(nc, tc_, t, ctx)


def _build_body_inner(nc, tc_, t, ctx):
    pool = ctx.enter_context
    con = pool(tc_.tile_pool(name="con", bufs=1))
    wp = pool(tc_.tile_pool(name="wp", bufs=1))       # weights, resident
    lhp = pool(tc_.tile_pool(name="lhp", bufs=8))
    ohp = pool(tc_.tile_pool(name="ohp", bufs=4))
    xp = pool(tc_.tile_pool(name="xp", bufs=1))       # x/x2/x3aug/z/z2/t1/y/g
    sqp = pool(tc_.tile_pool(name="sqp", bufs=2))
    qkp = pool(tc_.tile_pool(name="qkp", bufs=4))
    vtp = pool(tc_.tile_pool(name="vtp", bufs=1))
    exp_p = pool(tc_.tile_pool(name="exp_p", bufs=4))
    rows = pool(tc_.tile_pool(name="rows", bufs=8))
    bcp = pool(tc_.tile_pool(name="bcp", bufs=6))
    lmp = pool(tc_.tile_pool(name="lmp", bufs=4))
    tmp_p = pool(tc_.tile_pool(name="tmp_p", bufs=4))

    ps = pool(tc_.tile_pool(name="ps", bufs=2, space="PSUM"))       # big 6160B
    ps_s = pool(tc_.tile_pool(name="ps_s", bufs=2, space="PSUM"))   # small 1540B

    def mm(out, lhsT, rhs, start, stop, dr=True, nmax=512):
        n = rhs.shape[-1]
        for c0 in range(0, n, nmax):
            c1 = min(n, c0 + nmax)
            r = rhs[:, :, c0:c1] if dr else rhs[:, c0:c1]
            nc.tensor.matmul(out[:, c0:c1], lhsT=lhsT, rhs=r,
                             start=start, stop=stop,
                             perf_mode=DR if dr else None)

    # ---------------- constants
    ones8_t = con.tile([128, 2, 16], FP8)
    nc.gpsimd.memset(ones8_t[:], 1.0)
    ones8 = ones8_t[:, :, 0:1]
    ones_colf = con.tile([128, 1], F32)
    nc.gpsimd.memset(ones_colf[:], 1.0)
    ones1_row = con.tile([1, 128], BF16)
    nc.gpsimd.memset(ones1_row[:], 1.0)
    ones_rowT = con.tile([1, T], BF16)
    nc.gpsimd.memset(ones_rowT[:], 1.0)

    iota_w = con.tile([128, W], I32)
    nc.gpsimd.iota(iota_w[:], pattern=[[1, W]], base=0, channel_multiplier=0)
    i385_i = tmp_p.tile([128, W + 1], I32, name="i385i", tag="tz", bufs=2)
    nc.gpsimd.iota(i385_i[:], pattern=[[1, W + 1]], base=0, channel_multiplier=0)
    iota385_f = con.tile([128, W + 1], F32)
    nc.vector.tensor_copy(iota385_f[:], i385_i[:])
    im1_i = tmp_p.tile([1, W + 1], I32, name="im1i", tag="tz", bufs=2)
    nc.gpsimd.iota(im1_i[:], pattern=[[1, W + 1]], base=-1, channel_multiplier=0)
    iotam1_f = con.tile([1, W + 1], F32)
    nc.vector.tensor_copy(iotam1_f[:], im1_i[:])
    iota_p = []
    for c in range(3):
        ip_i = tmp_p.tile([128, 1], I32, name=f"ipi{c}", tag="ipi", bufs=1)
        nc.gpsimd.iota(ip_i[:], pattern=[[0, 1]], base=128 * c, channel_multiplier=1)
        ip_f = con.tile([128, 1], F32, name=f"ipf{c}", tag=f"ipf{c}")
        nc.vector.tensor_copy(ip_f[:], ip_i[:])
        iota_p.append(ip_f)

    NM12 = con.tile([128, NB * 3], F32)
    M12 = con.tile([128, NB * 3], F32)

    # ---------------- residual / activation tiles
    def triple(name, width=T):
        return [xp.tile([128, 2, width], FP8, name=f"{name}{p}", tag=f"{name}{p}")
                for p in range(KP)]

    x_t = triple("x")
    z_t = triple("z")
    x2_t = triple("x2")
    z2_t = z_t            # z dead after attention; reuse for z2
    x3_t = triple("x3", 1552)  # TA=1540 padded to 16-mult
    t1_t = x2_t           # x2 dead after ffn2 evac; reuse for t1
    y_t = [xp.tile([128, 2, T], FP8, name=f"y{p}", tag=f"y{p}") for p in range(4)]
    gx_t = [xp.tile([128, 2, T], FP8, name=f"g{m}", tag=f"g{m}") for m in range(1)]
    # g: 3 slots on y (dead after Wo), 3 on wqk (dead after attention), 2 fresh
    g_t = None  # assigned after wqk_t exists

    mx_f = [None] * NB
    gold_f = [None] * NB
    cneg16 = [None] * NB

    # ================ P0: segment-mean pool, per row ================
    for b in range(NB):
        wid_i = tmp_p.tile([128, 8], I32, name=f"wid{b}", tag="wid", bufs=2)
        nc.sync.dma_start(wid_i[:], t['wid'][b].rearrange("(c p) -> p c", p=128))
        mxi = tmp_p.tile([1, 1], I32, name=f"mxi{b}", tag="mxi", bufs=2)
        nc.sync.dma_start(mxi[:], t['wid'][b:b + 1, S - 1:S])
        mf = rows.tile([1, 1], F32, name=f"mxf{b}", tag="rrow", bufs=5)
        nc.vector.tensor_copy(mf[:], mxi[:])
        mx_f[b] = mf
        g_i = tmp_p.tile([128, 3], I32, name=f"gi{b}", tag="gi", bufs=2)
        nc.sync.dma_start(g_i[:], t['gold'][b].rearrange("(c p) -> p c", p=128))
        gf = con.tile([128, 3], F32, name=f"goldf{b}", tag=f"goldf{b}")
        nc.vector.tensor_copy(gf[:], g_i[:])
        gold_f[b] = gf

        lh_t, oh_t = [], []
        for sp in range(SP):
            lh_ = lhp.tile([128, 2, D], FP8, name=f"lh{b}_{sp}", tag="lh")
            nc.sync.dma_start(lh_[:], t['lh'][b, sp])
            lh_t.append(lh_)
            oh_ = ohp.tile([128, 2, W], FP8, name=f"oh{b}_{sp}", tag="oh")
            for j in range(2):
                nc.vector.tensor_tensor(
                    out=oh_[:, j, :],
                    in0=wid_i[:, 2 * sp + j:2 * sp + j + 1].to_broadcast([128, W]),
                    in1=iota_w[:], op=ALU.is_equal)
            oh_t.append(oh_)

        cnts = ps_s.tile([1, 512], F32, name=f"cnts{b}", tag="ps_s")
        for sp in range(SP):
            nc.tensor.matmul(cnts[:, 0:W], lhsT=ones8, rhs=oh_t[sp][:],
                             start=(sp == 0), stop=(sp == SP - 1), perf_mode=DR)
        sums_a = ps.tile([128, 3, 512], F32, name=f"sumsa{b}", tag="ps")
        sums_b = ps.tile([128, 3, 512], F32, name=f"sumsb{b}", tag="ps")
        for d in range(6):
            dst = (sums_a if d < 3 else sums_b)[:, d % 3, 0:W]
            for sp in range(SP):
                nc.tensor.matmul(dst, lhsT=lh_t[sp][:, :, 128 * d:128 * (d + 1)],
                                 rhs=oh_t[sp][:], start=(sp == 0),
                                 stop=(sp == SP - 1), perf_mode=DR)

        c1 = rows.tile([1, W], F32, name=f"c1{b}", tag="rrow", bufs=5)
        nc.vector.tensor_scalar_max(c1[:], cnts[:, 0:W], 1.0)
        rcp = rows.tile([1, W], F32, name=f"rcp{b}", tag="rrow", bufs=5)
        nc.vector.reciprocal_approx_fast(out=rcp[:], in_=c1[:])
        rcp16 = rows.tile([1, W], F32, name=f"rcp16{b}", tag="rrow", bufs=5)
        nc.vector.tensor_scalar_mul(rcp16[:], rcp[:], SC)
        rb = bcp.tile([128, W], F32, name=f"rb{b}", tag="bc", bufs=3)
        nc.gpsimd.partition_broadcast(rb[:], rcp16[:])
        for d in range(6):
            src = (sums_a if d < 3 else sums_b)[:, d % 3, 0:W]
            nc.vector.tensor_tensor(
                out=x_t[d // 2][:, d % 2, W * b:W * (b + 1)],
                in0=src, in1=rb[:], op=ALU.mult)

        maxid = tmp_p.tile([128, 1], F32, name=f"maxid{b}", tag="maxid", bufs=2)
        nc.gpsimd.partition_broadcast(maxid[:], mf[:])
        for c in range(3):
            nc.vector.tensor_tensor(out=M12[:, 3 * b + c:3 * b + c + 1],
                                    in0=iota_p[c][:], in1=maxid[:], op=ALU.is_le)
        ct = rows.tile([1, W + 1], F32, name=f"ct{b}", tag="rrow", bufs=5)
        nc.vector.tensor_tensor(out=ct[:], in0=iotam1_f[:],
                                in1=mf[0:1, 0:1].to_broadcast([1, W + 1]),
                                op=ALU.is_gt)
        cn = rows.tile([1, W + 1], F32, name=f"cneg{b}", tag=f"cnegr{b}", bufs=1)
        nc.vector.tensor_scalar_mul(cn[:], ct[:], NEG16)
        cneg16[b] = cn

    # ---------------- weights (DMAs queue after lh)
    def wload(name, n, width, tag):
        ts = []
        for p in range(n):
            w_ = wp.tile([128, 2, width], FP8, name=f"{tag}{p}", tag=f"{tag}{p}")
            nc.sync.dma_start(w_[:], t[name][p])
            ts.append(w_)
        return ts

    wv_t = wload('wv', KP, 784, 'wv')
    wqk_t = wload('wqk', KP, 2048, 'wqk')
    g_t = [y_t[0][:, :, :], y_t[1][:, :, :], y_t[2][:, :, :], y_t[3][:, :, :],
           wqk_t[0][:, :, 0:T], wqk_t[1][:, :, 0:T], wqk_t[2][:, :, 0:T],
           gx_t[0][:, :, :]]
    wo_t = wload('wo', 4, D, 'wo')
    w1_t = wload('w1', KP, FF, 'w1')
    w2_t = []
    for p in range(FP):
        w_ = lhp.tile([128, 2, D], FP8, name=f"w2_{p}", tag="lh")
        nc.sync.dma_start(w_[:], t['w2'][p])
        w2_t.append(w_)
    wbi_t = wload('wbi', KP, D, 'wbi')
    uw_t = wload('uw', KP, 16, 'uw')

    bqk_c = con.tile([128, 16], F32)
    nc.sync.dma_start(bqk_c[:], t['bqk'][:, :])
    b1_c = con.tile([128, 16], F32)
    nc.sync.dma_start(b1_c[:], t['b1c'][:, :])
    bv_row = con.tile([1, 784], BF16)
    nc.sync.dma_start(bv_row[:], t['bv16'][:, :])
    bo_row = con.tile([1, D], BF16)
    nc.sync.dma_start(bo_row[:], t['bo16'][:, :])
    b2_row = con.tile([1, D], BF16)
    nc.sync.dma_start(b2_row[:], t['b216'][:, :])
    ub_t = con.tile([1, 1], F32)
    nc.sync.dma_start(ub_t[:], t['ub16'][:, :])

    # ================ layer-norm: stats + z ================
    def ln_z(src, dst, label):
        s1 = ps.tile([1, T], F32, name=f"s1{label}", tag="ps")
        for p in range(KP):
            mm(s1, ones8, src[p][:], start=(p == 0), stop=(p == KP - 1))
        s2 = ps.tile([1, T], F32, name=f"s2{label}", tag="ps")
        for p in range(KP):
            sq = sqp.tile([128, 2, T], FP8, name=f"sq{label}{p}", tag="sq", bufs=2)
            nc.scalar.activation(sq[:], src[p][:], AF.Square, scale=1.0 / SC)
            mm(s2, ones8, sq[:], start=(p == 0), stop=(p == KP - 1))
        m16 = rows.tile([1, T], BF16, name=f"m16{label}", tag="lnL", bufs=2)
        nc.vector.tensor_scalar_mul(m16[:], s1[:], 1.0 / D)
        m256 = rows.tile([1, T], BF16, name=f"m256{label}", tag="lnS", bufs=1)
        nc.vector.tensor_tensor(out=m256[:], in0=m16[:], in1=m16[:], op=ALU.mult)
        v256 = rows.tile([1, T], F32, name=f"v256{label}", tag="lnF", bufs=1)
        nc.vector.scalar_tensor_tensor(out=v256[:], in0=s2[:], scalar=256.0 / D,
                                       in1=m256[:], op0=ALU.mult, op1=ALU.subtract)
        nc.vector.tensor_scalar_add(v256[:], v256[:], 256e-5)
        rec = rows.tile([1, T], F32, name=f"rec{label}", tag="lnF2", bufs=1)
        nc.vector.reciprocal_approx_fast(out=rec[:], in_=v256[:])
        rstd = rows.tile([1, T], BF16, name=f"rstd{label}", tag="lnL", bufs=2)
        nc.scalar.activation(rstd[:], rec[:], AF.Sqrt)
        rstd_b = bcp.tile([128, T], BF16, name=f"rstdb{label}", tag="bcT", bufs=2)
        nc.gpsimd.partition_broadcast(rstd_b[:], rstd[:])
        m16_b = bcp.tile([128, T], BF16, name=f"m16b{label}", tag="bcT", bufs=2)
        nc.gpsimd.partition_broadcast(m16_b[:], m16[:])
        for p in range(KP):
            for j in range(2):
                tz = tmp_p.tile([128, T], FP8, name=f"tz{label}{p}{j}", tag="tz", bufs=2)
                nc.gpsimd.tensor_tensor(out=tz[:], in0=src[p][:, j, :],
                                        in1=m16_b[:], op=ALU.subtract)
                nc.vector.tensor_tensor(out=dst[p][:, j, :], in0=tz[:],
                                        in1=rstd_b[:], op=ALU.mult)

    ln_z(x_t, z_t, "A")
    if 'dbg_x' in t:
        nc.sync.dma_start(t['dbg_x'][:], x_t[0][:])
        nc.sync.dma_start(t['dbg_z'][:], z_t[0][:])

    # ================ V (per row, per token chunk) ================
    v_pair = [None] * NB
    v_last = [None] * NB
    for b in range(NB):
        vp_ = vtp.tile([128, 2, 784], FP8, name=f"vp{b}", tag=f"vp{b}")
        vl_ = vtp.tile([128, 784], FP8, name=f"vl{b}", tag=f"vl{b}")
        v_pair[b] = vp_
        v_last[b] = vl_
        for c in range(3):
            vps = ps.tile([128, 1024], F32, name=f"vps{b}{c}", tag="ps")
            tok = W * b + 128 * c
            for p in range(KP):
                mm(vps[:, 0:784], z_t[p][:, :, tok:tok + 128], wv_t[p][:],
                   start=(p == 0), stop=False)
            mm(vps[:, 0:784], ones1_row[:], bv_row[:], start=False, stop=True,
               dr=False)
            dst = vp_[:, c, :] if c < 2 else vl_[:]
            nc.scalar.copy(dst, vps[:, 0:784])

    for pr in range(4):
        nc.gpsimd.memset(y_t[pr][:], 0.0)
    if 'dbg_v' in t:
        nc.sync.dma_start(t['dbg_v'][:], v_pair[0][:])

    # ================ attention, per head ================
    for h in range(H):
        qk_sb = []
        for m in (h, H + h):
            qp = ps.tile([128, T], F32, name=f"qp{h}{m}", tag="ps")
            for p in range(KP):
                mm(qp, wqk_t[p][:, :, 128 * m:128 * (m + 1)], z_t[p][:],
                   start=(p == 0), stop=(p == KP - 1))
            qs = qkp.tile([128, T], FP8, name=f"qk{h}{m}", tag="qk", bufs=3)
            nc.scalar.activation(qs[:], qp[:], AF.Identity, scale=1.0 / SC,
                                 bias=bqk_c[:, m:m + 1])
            qk_sb.append(qs)
        q_sb, k_sb = qk_sb
        if h == 0 and 'dbg_q' in t:
            nc.sync.dma_start(t['dbg_q'][:], q_sb[:])
            nc.sync.dma_start(t['dbg_k'][:], k_sb[:])

        for b in range(NB):
            sp_ = ps.tile([128, 3, 512], F32, name=f"sp{h}{b}", tag="ps")
            for c in range(3):
                nc.tensor.matmul(sp_[:, c, 0:W],
                                 lhsT=k_sb[:, W * b + 128 * c:W * b + 128 * (c + 1)],
                                 rhs=q_sb[:, W * b:W * (b + 1)],
                                 start=True, stop=True)
            exp_ = exp_p.tile([128, 2, W], FP8, name=f"exp{h}{b}", tag="exp", bufs=2)
            exl_ = exp_p.tile([128, W], FP8, name=f"exl{h}{b}", tag="exl", bufs=2)
            nc.scalar.activation(exp_[:], sp_[:, 0:2, 0:W], AF.Exp)
            nc.scalar.activation(exl_[:], sp_[:, 2, 0:W], AF.Exp)
            if h == 0 and b == 0 and 'dbg_ex' in t:
                nc.sync.dma_start(t['dbg_ex'][:], exp_[:])

            yp = ps_s.tile([128, 512], F32, name=f"yp{h}{b}", tag="ps_s")
            nc.tensor.matmul(yp[0:97, 0:W], lhsT=v_pair[b][:, :, 97 * h:97 * h + 97],
                             rhs=exp_[:], start=True, stop=False, perf_mode=DR)
            nc.tensor.matmul(yp[0:97, 0:W], lhsT=v_last[b][:, 97 * h:97 * h + 97],
                             rhs=exl_[:], start=False, stop=True)
            dn = rows.tile([1, W], F32, name=f"dn{h}{b}", tag="rrow", bufs=5)
            nc.vector.tensor_copy(dn[:], yp[96:97, 0:W])
            rbr = rows.tile([1, W], F32, name=f"rbr{h}{b}", tag="rrow", bufs=5)
            nc.vector.reciprocal_approx_fast(out=rbr[:], in_=dn[:])
            rb_b = bcp.tile([128, W], F32, name=f"arb{h}{b}", tag="bc", bufs=3)
            nc.gpsimd.partition_broadcast(rb_b[:], rbr[:])
            nc.vector.tensor_tensor(
                out=y_t[h // 2][0:96, h % 2, W * b:W * (b + 1)],
                in0=yp[0:96, 0:W], in1=rb_b[0:96, :], op=ALU.mult)

    # ================ Wo + residual ================
    for m in range(6):
        op_ = ps.tile([128, T], F32, name=f"wops{m}", tag="ps")
        for p in range(4):
            mm(op_, wo_t[p][:, :, 128 * m:128 * (m + 1)], y_t[p][:],
               start=(p == 0), stop=False)
        mm(op_, bo_row[:, 128 * m:128 * (m + 1)], ones_rowT[:],
           start=False, stop=True, dr=False)
        nc.vector.tensor_tensor(out=x2_t[m // 2][:, m % 2, :], in0=op_[:],
                                in1=x_t[m // 2][:, m % 2, :], op=ALU.add)

    if 'dbg_y' in t:
        nc.sync.dma_start(t['dbg_y'][:], y_t[0][:])
        nc.sync.dma_start(t['dbg_x2'][:], x2_t[0][:])
    ln_z(x2_t, z2_t, "B")

    # ================ FFN ================
    for m in range(16):
        wp_ = ps.tile([128, T], F32, name=f"ffps{m}", tag="ps")
        for p in range(KP):
            mm(wp_, w1_t[p][:, :, 128 * m:128 * (m + 1)], z2_t[p][:],
               start=(p == 0), stop=(p == KP - 1))
        nc.scalar.activation(g_t[m // 2][:, m % 2, :], wp_[:], AF.Gelu,
                             scale=1.0 / SC, bias=b1_c[:, m:m + 1])

    # root cols into x3 before the evacs
    for p in range(KP):
        for b in range(NB):
            nc.sync.dma_start(x3_t[p][:, :, (W + 1) * b:(W + 1) * b + 1],
                              t['root'][p])

    for m in range(6):
        fp_ = ps.tile([128, T], F32, name=f"f2ps{m}", tag="ps")
        for p in range(FP):
            mm(fp_, w2_t[p][:, :, 128 * m:128 * (m + 1)], g_t[p][:],
               start=(p == 0), stop=False)
        mm(fp_, b2_row[:, 128 * m:128 * (m + 1)], ones_rowT[:],
           start=False, stop=True, dr=False)
        for b in range(NB):
            nc.vector.tensor_tensor(
                out=x3_t[m // 2][:, m % 2, (W + 1) * b + 1:(W + 1) * (b + 1)],
                in0=fp_[:, W * b:W * (b + 1)],
                in1=x2_t[m // 2][:, m % 2, W * b:W * (b + 1)], op=ALU.add)

    # ================ biaffine t1 + u ================
    for m in range(6):
        t1ps = ps.tile([128, T], F32, name=f"t1ps{m}", tag="ps")
        for b in range(NB):
            for p in range(KP):
                nc.tensor.matmul(
                    t1ps[:, W * b:W * (b + 1)],
                    lhsT=wbi_t[p][:, :, 128 * m:128 * (m + 1)],
                    rhs=x3_t[p][:, :, (W + 1) * b + 1:(W + 1) * (b + 1)],
                    start=(p == 0), stop=(p == KP - 1), perf_mode=DR)
        nc.scalar.activation(t1_t[m // 2][:, m % 2, :], t1ps[:], AF.Identity,
                             scale=1.0 / 256.0)

    if 'dbg_x3' in t:
        nc.sync.dma_start(t['dbg_x3'][:], x3_t[0][:])
        nc.sync.dma_start(t['dbg_t1'][:], t1_t[0][:])
    u16 = rows.tile([1, TA], F32, name="u16", tag="u16", bufs=1)
    for b in range(NB):
        upx = ps_s.tile([1, 512], F32, name=f"upx{b}", tag="ps_s")
        for p in range(KP):
            nc.tensor.matmul(upx[:, 0:W + 1], lhsT=uw_t[p][:, :, 0:1],
                             rhs=x3_t[p][:, :, (W + 1) * b:(W + 1) * (b + 1)],
                             start=(p == 0), stop=(p == KP - 1), perf_mode=DR)
        nc.scalar.activation(u16[:, (W + 1) * b:(W + 1) * (b + 1)],
                             upx[:, 0:W + 1], AF.Identity, scale=1.0 / SC,
                             bias=ub_t[0:1, 0:1])
    if 'dbg_u' in t:
        nc.sync.dma_start(t['dbg_u'][:], u16[:])
    cneg_b = []
    for b in range(NB):
        cu = rows.tile([1, W + 1], F32, name=f"cu{b}", tag="rrow", bufs=5)
        nc.vector.tensor_tensor(out=cu[:], in0=cneg16[b][:],
                                in1=u16[:, (W + 1) * b:(W + 1) * (b + 1)],
                                op=ALU.add)
        cb = bcp.tile([128, W + 1], F32, name=f"cub{b}", tag=f"cub{b}", bufs=1)
        nc.gpsimd.partition_broadcast(cb[:], cu[:])
        cneg_b.append(cb)

    # ================ logits + loss ================
    for b in range(NB):
        for c in range(3):
            L = ps_s.tile([128, 512], F32, name=f"L{b}{c}", tag="ps_s")
            for p in range(KP):
                nc.tensor.matmul(
                    L[:, 0:W + 1],
                    lhsT=t1_t[p][:, :, W * b + 128 * c:W * b + 128 * (c + 1)],
                    rhs=x3_t[p][:, :, (W + 1) * b:(W + 1) * (b + 1)],
                    start=(p == 0), stop=(p == KP - 1), perf_mode=DR)
            Lm = lmp.tile([128, W + 1], F32, name=f"Lm{b}{c}", tag="lm", bufs=3)
            nc.vector.tensor_tensor(out=Lm[:], in0=L[:, 0:W + 1], in1=cneg_b[b][:],
                                    op=ALU.add)
            if b == 0 and c == 0 and 'dbg_lm' in t:
                nc.sync.dma_start(t['dbg_lm'][:], Lm[:])
            nmx = rows.tile([128, 1], F32, name=f"nmx{b}{c}", tag="colf", bufs=12)
            nc.vector.tensor_reduce(out=nmx[:], in_=Lm[:], axis=AX.X, op=ALU.max,
                                    negate=True)
            nmxs = rows.tile([128, 1], F32, name=f"nmxs{b}{c}", tag="colf", bufs=12)
            nc.vector.tensor_scalar_mul(nmxs[:], nmx[:], 1.0 / SC)
            E = lmp.tile([128, W + 1], FP8, name=f"E{b}{c}", tag="e8", bufs=1)
            Ssum = rows.tile([128, 1], F32, name=f"S{b}{c}", tag="colf", bufs=12)
            nc.scalar.activation(E[:], Lm[:], AF.Exp, scale=1.0 / SC,
                                 bias=nmxs[:], accum_out=Ssum[:])
            lnS = rows.tile([128, 1], F32, name=f"lnS{b}{c}", tag="colf", bufs=12)
            nc.scalar.activation(lnS[:], Ssum[:], AF.Ln)
            oneh = lmp.tile([128, W + 1], F32, name=f"oneh{b}{c}", tag="lm", bufs=3)
            nc.vector.tensor_tensor(
                out=oneh[:], in0=iota385_f[:],
                in1=gold_f[b][:, c:c + 1].to_broadcast([128, W + 1]),
                op=ALU.is_equal)
            E2 = lmp.tile([128, W + 1], F32, name=f"E2{b}{c}", tag="lm", bufs=3)
            nc.vector.tensor_tensor(out=E2[:], in0=Lm[:], in1=oneh[:], op=ALU.mult)
            picked = rows.tile([128, 1], F32, name=f"pk{b}{c}", tag="colf", bufs=12)
            nc.vector.tensor_reduce(out=picked[:], in_=E2[:], axis=AX.X, op=ALU.add)
            pk2 = rows.tile([128, 1], F32, name=f"pk2{b}{c}", tag="colf", bufs=12)
            nc.vector.scalar_tensor_tensor(out=pk2[:], in0=picked[:],
                                           scalar=1.0 / SC, in1=nmxs[:],
                                           op0=ALU.mult, op1=ALU.add)
            nll = rows.tile([128, 1], F32, name=f"nll{b}{c}", tag="colf", bufs=12)
            nc.vector.tensor_tensor(out=nll[:], in0=lnS[:], in1=pk2[:],
                                    op=ALU.subtract)
            j = 3 * b + c
            nc.vector.tensor_tensor(out=NM12[:, j:j + 1], in0=nll[:],
                                    in1=M12[:, j:j + 1], op=ALU.mult)

    # ================ final reduction ================
    if 'dbg_nm' in t:
        nc.sync.dma_start(t['dbg_nm'][:], NM12[:])
    out_sb = con.tile([1, 2], F32)
    fp1 = ps_s.tile([1, 512], F32, name="fin1", tag="ps_s")
    nc.tensor.matmul(fp1[:, 0:NB * 3], lhsT=ones_colf[:], rhs=NM12[:],
                     start=True, stop=True)
    nc.vector.tensor_reduce(out=out_sb[:, 0:1], in_=fp1[:, 0:NB * 3], axis=AX.X,
                            op=ALU.add)
    fp2 = ps_s.tile([1, 512], F32, name="fin2", tag="ps_s")
    nc.tensor.matmul(fp2[:, 0:NB * 3], lhsT=ones_colf[:], rhs=M12[:],
                     start=True, stop=True)
    nc.vector.tensor_reduce(out=out_sb[:, 1:2], in_=fp2[:, 0:NB * 3], axis=AX.X,
                            op=ALU.add)
    nc.sync.dma_start(t['out'][:, :], out_sb[:])


# ---------------------------------------------------------------- driver

_CACHE = {}


def build_nc():
    if 'nc' in _CACHE:
        return _CACHE['nc']
    nc = bacc.Bacc("TRN2", target_bir_lowering=False, debug=False)
    t = _declare(nc)
    with tile.TileContext(nc) as tc_:
        _build_body(nc, tc_, t)
    nc.compile()
    _CACHE['nc'] = nc
    return nc


def kernel(**inputs):
    nc = build_nc()
    in_maps = make_in_maps(inputs)
    res = run_bass_kernel_spmd(nc, in_maps, core_ids=list(range(NCORES)))
    num = 0.0
    den = 0.0
    for c in range(NCORES):
        o = res.results[c]['out']
        num += float(o[0, 0])
        den += float(o[0, 1])
    return np.float32(num / den)


if __name__ == '__main__':
    build_nc()
    print("build + compile OK")
